# revision 36
# baseline (speedup 1.0000x reference)
"""Trainium2 Bass kernel for nn_ConvDY2d (dynamic-weight 3x3 conv, CondConv-style).

Reference computation (B=16, C=O=256, H=W=64, K=4 mixing kernels):
  attn  = softmax(MLP(global_avg_pool(x)) / 30)            # [B, 4]
  w_mix = einsum('bk,koihw->boihw', attn, w_dyn)           # per-sample 3x3 conv kernel
  out[b] = conv2d(x[b], w_mix[b], padding=1)

Strategy: data-parallel over batch, 2 samples per NeuronCore across 8 cores.
Per core, the conv is an implicit GEMM: for each (out-channel block, 8-row
group) a [128, 512] PSUM tile accumulates 18 float32r matmuls (2 c-blocks x
9 taps) whose rhs are contiguous 512-element slices of a row-padded input
image ([128c, 4226]: 66 rows x 64 cols + 1 elem pad on each end).  Column
wrap-around at row edges is fixed up afterwards by subtracting per-border
correction terms computed with 12 small strided-rhs matmuls per output block.
Attention MLP + softmax + weight mixing all run on-device per sample.
"""

import sys

if "/opt/trn_rl_repo" not in sys.path:
    sys.path.insert(0, "/opt/trn_rl_repo")

import numpy as np

B, C, H, W = 16, 256, 64, 64
O, K, KS = 256, 4, 3
MID = C // 4
INV_DELTA = 1.0 / 30.0
NCORES = 8
NB = B // NCORES            # samples per core
NPOS = KS * KS              # 9 taps
FPAD = 1 + 66 * W + 1       # padded image free size: 4226
ROW0 = 65                   # flat offset of input row 0 (= 1 + 1*64)

_CACHE = {}


def _build_nc():
    import concourse.bacc as bacc
    import concourse.tile as tile
    from concourse import mybir
    from concourse.tile_rust import add_dep_helper

    f32 = mybir.dt.float32
    f32r = mybir.dt.float32r
    AX = mybir.AxisListType
    ALU = mybir.AluOpType
    ACTF = mybir.ActivationFunctionType

    nc = bacc.Bacc(target_bir_lowering=False, debug=False)

    bf16 = mybir.dt.bfloat16

    # Conv datapath runs in bf16 (host pre-converts x / w_dyn): full-rate PE
    # with FWL weight loads and half the DMA bytes of fp32.  Accumulation is
    # fp32 in PSUM; MLP/softmax/corrections all stay fp32.
    x_d = nc.dram_tensor("x", [NB, C, H, W], bf16, kind="ExternalInput").ap()
    wd_d = nc.dram_tensor("wdynT", [K, NPOS, C, O], bf16, kind="ExternalInput").ap()
    fc1wT_d = nc.dram_tensor("fc1wT", [C, MID], f32, kind="ExternalInput").ap()
    fc1b_d = nc.dram_tensor("fc1b", [1, MID], f32, kind="ExternalInput").ap()
    fc2aug_d = nc.dram_tensor("fc2aug", [MID + 1, K], f32, kind="ExternalInput").ap()
    out_d = nc.dram_tensor("out", [NB, O, H, W], f32, kind="ExternalOutput").ap()

    with tile.TileContext(nc) as tc:
        with (
            tc.tile_pool(name="consts", bufs=1) as constp,
            tc.tile_pool(name="wdyn", bufs=1) as wdynp,
            tc.tile_pool(name="wmix", bufs=1) as wmixp,
            tc.tile_pool(name="xpad", bufs=1) as xpadp,
            tc.tile_pool(name="osb", bufs=4) as osbp,
            tc.tile_pool(name="convps", bufs=5, space="PSUM") as convps,
            tc.tile_pool(name="corrps", bufs=2, space="PSUM") as corrps,
            tc.tile_pool(name="smallps", bufs=1, space="PSUM") as smallps,
        ):
            # ---------------- tiny consts FIRST (ahead of bulk DMA) ----------------
            fc1wT_sb = constp.tile([128, 2 * MID], f32, tag="fc1w", name="fc1wT_sb")
            for cb in range(2):
                nc.sync.dma_start(
                    fc1wT_sb[:, cb * MID : (cb + 1) * MID],
                    fc1wT_d[cb * 128 : (cb + 1) * 128, :],
                )
            fc1b_sb = constp.tile([1, MID], f32, tag="fc1b", name="fc1b_sb")
            nc.sync.dma_start(fc1b_sb, fc1b_d)
            fc2aug_sb = constp.tile([MID + 1, K], f32, tag="fc2", name="fc2aug_sb")
            nc.sync.dma_start(fc2aug_sb, fc2aug_d)
            ones_sb = constp.tile([1, 128], f32, tag="ones", name="ones_sb")
            nc.gpsimd.memset(ones_sb, 1.0)

            # ---------------- bulk loads: x[b0] -> wdyn(cb0) -> wdyn(cb1) ----------
            # x DMAs are split in row-halves so pooling partial-reduces start
            # while the second half is still in flight.
            xpad = [[None, None] for _ in range(NB)]

            NCHUNK = 4

            def load_x(b):
                rows = H // NCHUNK
                for cb in range(2):
                    t = xpadp.tile([128, FPAD], bf16, tag=f"xpad{b}{cb}", name=f"xpad{b}{cb}")
                    nc.gpsimd.memset(t[:, 0:ROW0], 0.0)
                    nc.gpsimd.memset(t[:, ROW0 + H * W : FPAD], 0.0)
                    xpad[b][cb] = t
                # interleave c-block chunks so both pooling engines (DVE on
                # cb0, ACT on cb1) receive data concurrently
                for h in range(NCHUNK):
                    for cb in range(2):
                        nc.sync.dma_start(
                            xpad[b][cb][:, ROW0 + h * rows * W : ROW0 + (h + 1) * rows * W],
                            x_d[
                                b, cb * 128 : (cb + 1) * 128, h * rows : (h + 1) * rows, :
                            ].rearrange("c h w -> c (h w)"),
                        )

            load_x(0)

            wdyn = [[None, None] for _ in range(K)]
            for cb in range(2):
                for k in range(K):
                    t = wdynp.tile([128, NPOS * O], bf16, tag=f"wd{k}{cb}", name=f"wd{k}{cb}")
                    nc.sync.dma_start(
                        t.rearrange("c (p o) -> c p o", o=O),
                        wd_d[k, :, cb * 128 : (cb + 1) * 128, :].transpose([1, 0, 2]),
                    )
                    wdyn[k][cb] = t

            # ---------------- per-sample attention -> mixed weights ----------------
            wmix = [[None, None] for _ in range(NB)]

            act_dummy = constp.tile([128, H // NCHUNK * W], bf16, tag="actdum", name="act_dummy")

            def attn_and_mix(b):
                # global sum pool (mean scale folded into fc1wT host-side);
                # partial reduces per c-block chase the DMA row-chunks.
                # cb0 reduces on DVE, cb1 on ACT (Copy with accum_out) so the
                # two c-blocks pool in parallel.
                pooled = [None, None]
                chunk = H // NCHUNK * W
                for cb in range(2):
                    pp = constp.tile([128, NCHUNK], f32, tag=f"pp{b}{cb}", name=f"pp{b}{cb}")
                    for h in range(NCHUNK):
                        src = xpad[b][cb][:, ROW0 + h * chunk : ROW0 + (h + 1) * chunk]
                        if cb == 0:
                            nc.vector.reduce_sum(pp[:, h : h + 1], src, AX.X)
                        else:
                            nc.scalar.activation(
                                act_dummy, src, ACTF.Copy, accum_out=pp[:, h : h + 1]
                            )
                    p = constp.tile([128, 1], f32, tag=f"pool{b}{cb}", name=f"pooled{b}{cb}")
                    nc.vector.reduce_sum(p, pp, AX.X)
                    pooled[cb] = p

                hid_ps = smallps.tile([MID, 1], f32, tag="small", name=f"hid_ps{b}")
                for cb in range(2):
                    nc.tensor.matmul(
                        hid_ps,
                        fc1wT_sb[:, cb * MID : (cb + 1) * MID],
                        pooled[cb],
                        start=(cb == 0),
                        stop=False,
                    )
                nc.tensor.matmul(
                    hid_ps, fc1b_sb, ones_sb[:, 0:1], start=False, stop=True
                )

                hid_sb = constp.tile([MID + 1, 1], f32, tag=f"hid{b}", name=f"hid_sb{b}")
                nc.gpsimd.memset(hid_sb[MID : MID + 1, :], 1.0)
                nc.scalar.activation(hid_sb[0:MID, :], hid_ps, ACTF.Relu)

                lg_ps = smallps.tile([1, K], f32, tag="small", name=f"lg_ps{b}")
                nc.tensor.matmul(lg_ps, hid_sb, fc2aug_sb, start=True, stop=True)

                # softmax without max-shift (|logits|/30 is small) and with the
                # denominator accumulated inside the Exp activation
                ex = constp.tile([1, K], f32, tag=f"ex{b}", name=f"ex{b}")
                sm = constp.tile([1, 1], f32, tag=f"sm{b}", name=f"sm{b}")
                nc.scalar.activation(ex, lg_ps, ACTF.Exp, accum_out=sm)
                rc = constp.tile([1, 1], f32, tag=f"rc{b}", name=f"rc{b}")
                nc.vector.reciprocal(rc, sm)
                attn = constp.tile([1, K], f32, tag=f"at{b}", name=f"attn{b}")
                nc.vector.tensor_scalar_mul(attn, ex, rc)

                # broadcast attn row to all 128 partitions (gpsimd ISA op)
                attn_bc = constp.tile([128, K], f32, tag=f"abc{b}", name=f"attn_bc{b}")
                nc.gpsimd.partition_broadcast(attn_bc, attn)

                # mixed transposed weights: [128c, pos*256 + o].  Accumulate the
                # first K-1 terms into an fp32 scratch (single bf16 rounding at
                # the end instead of one per step).  Force the cb0 chain to
                # finish before cb1 starts so cb0 conv matmuls unblock at
                # half-time instead of both chains finishing together.
                # Four sub-chains (cb x free-half), strictly ordered on DVE so
                # each finished half releases its conv matmuls (subtile deps)
                # at quarter-points instead of everything at the end.
                HALF = 5 * O  # pos 0-4 first (covers the earliest-emitted MMs)
                prev_last = None
                for cb in range(2):
                    sc = wmixp.tile([128, NPOS * O], f32, tag="mixsc", bufs=2, name=f"mixsc{b}{cb}")
                    wm = wmixp.tile(
                        [128, NPOS * O], bf16, tag=f"wm{b}{cb}", name=f"wmix{b}{cb}"
                    )
                    for lo, hi in ((0, HALF), (HALF, NPOS * O)):
                        first = nc.vector.tensor_scalar_mul(
                            sc[:, lo:hi], wdyn[0][cb][:, lo:hi], attn_bc[:, 0:1]
                        )
                        if prev_last is not None:
                            add_dep_helper(
                                first.ins,
                                prev_last.ins,
                                sync=False,
                                reason="mix sub-chain ordering",
                            )
                        for k in range(1, K - 1):
                            nc.vector.scalar_tensor_tensor(
                                sc[:, lo:hi],
                                wdyn[k][cb][:, lo:hi],
                                attn_bc[:, k : k + 1],
                                sc[:, lo:hi],
                                op0=ALU.mult,
                                op1=ALU.add,
                            )
                        prev_last = nc.vector.scalar_tensor_tensor(
                            wm[:, lo:hi],
                            wdyn[K - 1][cb][:, lo:hi],
                            attn_bc[:, K - 1 : K],
                            sc[:, lo:hi],
                            op0=ALU.mult,
                            op1=ALU.add,
                        )
                    wmix[b][cb] = wm

            attn_and_mix(0)
            load_x(1)
            attn_and_mix(1)

            # ---------------- main conv ----------------
            def wsl(b, cb, pos, ob):
                off = pos * O + ob * 128
                return wmix[b][cb][:, off : off + 128]

            for b in range(NB):
                for ob in range(2):
                    # border corrections: [128o, side*64 + y]
                    corr = corrps.tile([128, 128], f32, tag="corr", name=f"corr{b}{ob}")

                    def emit_corr():
                        for side, dxv in ((0, 0), (1, 2)):
                            i = 0
                            for cb in range(2):
                                for dy in range(KS):
                                    s = dy * W + (0 if side == 0 else ROW0)
                                    rhs = xpad[b][cb][:, s : s + (H - 1) * W + 1 : W]
                                    nc.tensor.matmul(
                                        corr[:, side * 64 : side * 64 + 64],
                                        wsl(b, cb, dy * KS + dxv, ob),
                                        rhs,
                                        start=(i == 0),
                                        stop=(i == 5),
                                    )
                                    i += 1

                    for rg in range(8):
                        y0 = rg * 8
                        cps = convps.tile([128, 512], f32, tag="conv", name=f"cps{b}{ob}{rg}")
                        i = 0
                        for cb in range(2):
                            for dy in range(KS):
                                for dx in range(KS):
                                    s = (y0 + dy) * W + dx
                                    nc.tensor.matmul(
                                        cps,
                                        wsl(b, cb, dy * KS + dx, ob),
                                        xpad[b][cb][:, s : s + 512],
                                        start=(i == 0),
                                        stop=(i == 17),
                                    )
                                    i += 1
                        if rg == 0:
                            # corr emitted after the first rg group: conv
                            # matmuls outrank it while cb1 mixing trails
                            emit_corr()
                        osb = osbp.tile([128, 512], f32, tag="osb", name=f"osb{b}{ob}{rg}")
                        nc.scalar.copy(osb, cps)
                        # subtract wrap-around bleed on columns 0 and 63
                        ov = osb.rearrange("m (y x) -> m y x", x=W)[:, :, 0 : W : W - 1]
                        cv = corr.rearrange("m (s y) -> m y s", s=2)[:, y0 : y0 + 8, :]
                        nc.vector.tensor_sub(ov, ov, cv)
                        nc.sync.dma_start(
                            out_d[b, ob * 128 : (ob + 1) * 128, y0 : y0 + 8, :],
                            osb.rearrange("m (y x) -> m y x", x=W),
                        )

    nc.compile()
    return nc


def get_nc():
    if "nc" not in _CACHE:
        _CACHE["nc"] = _build_nc()
    return _CACHE["nc"]


def prep_inputs(x, w_dyn, fc1_w, fc1_b, fc2_w, fc2_b):
    """Host-side layout prep + batch sharding -> per-core input maps."""
    import ml_dtypes

    bf16 = ml_dtypes.bfloat16
    w_dynT = np.ascontiguousarray(
        np.transpose(np.asarray(w_dyn, np.float32), (0, 3, 4, 2, 1)).reshape(K, NPOS, C, O)
    ).astype(bf16)
    fc1wT = np.ascontiguousarray(np.asarray(fc1_w, np.float32).T) / float(H * W)
    fc1b = np.ascontiguousarray(np.asarray(fc1_b, np.float32).reshape(1, MID))
    fc2aug = np.ascontiguousarray(
        np.vstack([np.asarray(fc2_w, np.float32).T, np.asarray(fc2_b, np.float32)[None, :]])
        * INV_DELTA
    )
    x = np.asarray(x, np.float32).astype(bf16)
    in_maps = []
    for core in range(NCORES):
        in_maps.append(
            {
                "x": np.ascontiguousarray(x[core * NB : (core + 1) * NB]),
                "wdynT": w_dynT,
                "fc1wT": fc1wT,
                "fc1b": fc1b,
                "fc2aug": fc2aug,
            }
        )
    return in_maps


def kernel(x, w_dyn, fc1_w, fc1_b, fc2_w, fc2_b):
    from concourse.bass_utils import run_bass_kernel_spmd

    nc = get_nc()
    in_maps = prep_inputs(x, w_dyn, fc1_w, fc1_b, fc2_w, fc2_b)
    res = run_bass_kernel_spmd(nc, in_maps, core_ids=list(range(NCORES)))
    return np.concatenate([r["out"] for r in res.results], axis=0)


# revision 37
# speedup vs baseline: 1.2077x; 1.2077x over previous
"""Trainium2 Bass kernel for nn_ConvDY2d (dynamic-weight 3x3 conv, CondConv-style).

Reference computation (B=16, C=O=256, H=W=64, K=4 mixing kernels):
  attn  = softmax(MLP(global_avg_pool(x)) / 30)            # [B, 4]
  w_mix = einsum('bk,koihw->boihw', attn, w_dyn)           # per-sample 3x3 conv kernel
  out[b] = conv2d(x[b], w_mix[b], padding=1)

Strategy: data-parallel over batch, 2 samples per NeuronCore across 8 cores.
Per core, the conv is an implicit GEMM: for each (out-channel block, 8-row
group) a [128, 512] PSUM tile accumulates 18 float32r matmuls (2 c-blocks x
9 taps) whose rhs are contiguous 512-element slices of a row-padded input
image ([128c, 4226]: 66 rows x 64 cols + 1 elem pad on each end).  Column
wrap-around at row edges is fixed up afterwards by subtracting per-border
correction terms computed with 12 small strided-rhs matmuls per output block.
Attention MLP + softmax + weight mixing all run on-device per sample.
"""

import sys

if "/opt/trn_rl_repo" not in sys.path:
    sys.path.insert(0, "/opt/trn_rl_repo")

import numpy as np

B, C, H, W = 16, 256, 64, 64
O, K, KS = 256, 4, 3
MID = C // 4
INV_DELTA = 1.0 / 30.0
NCORES = 8
NB = B // NCORES            # samples per core
NPOS = KS * KS              # 9 taps
FPAD = 1 + 66 * W + 1       # padded image free size: 4226
ROW0 = 65                   # flat offset of input row 0 (= 1 + 1*64)

_CACHE = {}


def _build_nc():
    import concourse.bacc as bacc
    import concourse.tile as tile
    from concourse import mybir
    from concourse.tile_rust import add_dep_helper

    f32 = mybir.dt.float32
    f32r = mybir.dt.float32r
    AX = mybir.AxisListType
    ALU = mybir.AluOpType
    ACTF = mybir.ActivationFunctionType

    nc = bacc.Bacc(target_bir_lowering=False, debug=False)

    bf16 = mybir.dt.bfloat16

    # Conv datapath runs in bf16 (host pre-converts x / w_dyn): full-rate PE
    # with FWL weight loads and half the DMA bytes of fp32.  Accumulation is
    # fp32 in PSUM; MLP/softmax/corrections all stay fp32.
    x_d = nc.dram_tensor("x", [NB, C, H, W], bf16, kind="ExternalInput").ap()
    wd_d = nc.dram_tensor("wdynT", [K, NPOS, C, O], bf16, kind="ExternalInput").ap()
    fc1wT_d = nc.dram_tensor("fc1wT", [C, MID], f32, kind="ExternalInput").ap()
    fc1b_d = nc.dram_tensor("fc1b", [1, MID], f32, kind="ExternalInput").ap()
    fc2aug_d = nc.dram_tensor("fc2aug", [MID + 1, K], f32, kind="ExternalInput").ap()
    out_d = nc.dram_tensor("out", [NB, O, H, W], f32, kind="ExternalOutput").ap()

    with tile.TileContext(nc) as tc:
        with (
            tc.tile_pool(name="consts", bufs=1) as constp,
            tc.tile_pool(name="wdyn", bufs=1) as wdynp,
            tc.tile_pool(name="wmix", bufs=1) as wmixp,
            tc.tile_pool(name="xpad", bufs=1) as xpadp,
            tc.tile_pool(name="osb", bufs=4) as osbp,
            tc.tile_pool(name="convps", bufs=5, space="PSUM") as convps,
            tc.tile_pool(name="corrps", bufs=2, space="PSUM") as corrps,
            tc.tile_pool(name="smallps", bufs=1, space="PSUM") as smallps,
        ):
            # ---------------- tiny consts FIRST (ahead of bulk DMA) ----------------
            fc1wT_sb = constp.tile([128, 2 * MID], f32, tag="fc1w", name="fc1wT_sb")
            for cb in range(2):
                nc.sync.dma_start(
                    fc1wT_sb[:, cb * MID : (cb + 1) * MID],
                    fc1wT_d[cb * 128 : (cb + 1) * 128, :],
                )
            fc1b_sb = constp.tile([1, MID], f32, tag="fc1b", name="fc1b_sb")
            nc.sync.dma_start(fc1b_sb, fc1b_d)
            fc2aug_sb = constp.tile([MID + 1, K], f32, tag="fc2", name="fc2aug_sb")
            nc.sync.dma_start(fc2aug_sb, fc2aug_d)
            ones_sb = constp.tile([1, 128], f32, tag="ones", name="ones_sb")
            nc.gpsimd.memset(ones_sb, 1.0)

            # ---------------- bulk loads: x[b0] -> wdyn(cb0) -> wdyn(cb1) ----------
            # x DMAs are split in row-halves so pooling partial-reduces start
            # while the second half is still in flight.
            xpad = [[None, None] for _ in range(NB)]

            NCHUNK = 4

            def load_x(b):
                rows = H // NCHUNK
                for cb in range(2):
                    t = xpadp.tile([128, FPAD], bf16, tag=f"xpad{b}{cb}", name=f"xpad{b}{cb}")
                    nc.gpsimd.memset(t[:, 0:ROW0], 0.0)
                    nc.gpsimd.memset(t[:, ROW0 + H * W : FPAD], 0.0)
                    xpad[b][cb] = t
                # interleave c-block chunks so both pooling engines (DVE on
                # cb0, ACT on cb1) receive data concurrently
                for h in range(NCHUNK):
                    for cb in range(2):
                        nc.sync.dma_start(
                            xpad[b][cb][:, ROW0 + h * rows * W : ROW0 + (h + 1) * rows * W],
                            x_d[
                                b, cb * 128 : (cb + 1) * 128, h * rows : (h + 1) * rows, :
                            ].rearrange("c h w -> c (h w)"),
                        )

            load_x(0)

            wdyn = [[None, None] for _ in range(K)]
            for cb in range(2):
                for k in range(K):
                    t = wdynp.tile([128, NPOS * O], bf16, tag=f"wd{k}{cb}", name=f"wd{k}{cb}")
                    nc.sync.dma_start(
                        t.rearrange("c (p o) -> c p o", o=O),
                        wd_d[k, :, cb * 128 : (cb + 1) * 128, :].transpose([1, 0, 2]),
                    )
                    wdyn[k][cb] = t

            # ---------------- per-sample attention -> mixed weights ----------------
            wmix = [[None, None] for _ in range(NB)]

            act_dummy = constp.tile([128, H // NCHUNK * W], bf16, tag="actdum", name="act_dummy")

            def attn_and_mix(b):
                # global sum pool (mean scale folded into fc1wT host-side);
                # partial reduces per c-block chase the DMA row-chunks.
                # cb0 reduces on DVE, cb1 on ACT (Copy with accum_out) so the
                # two c-blocks pool in parallel.
                pooled = [None, None]
                chunk = H // NCHUNK * W
                for cb in range(2):
                    pp = constp.tile([128, NCHUNK], f32, tag=f"pp{b}{cb}", name=f"pp{b}{cb}")
                    for h in range(NCHUNK):
                        src = xpad[b][cb][:, ROW0 + h * chunk : ROW0 + (h + 1) * chunk]
                        if cb == 0:
                            nc.vector.reduce_sum(pp[:, h : h + 1], src, AX.X)
                        else:
                            nc.scalar.activation(
                                act_dummy, src, ACTF.Copy, accum_out=pp[:, h : h + 1]
                            )
                    p = constp.tile([128, 1], f32, tag=f"pool{b}{cb}", name=f"pooled{b}{cb}")
                    nc.vector.reduce_sum(p, pp, AX.X)
                    pooled[cb] = p

                hid_ps = smallps.tile([MID, 1], f32, tag="small", name=f"hid_ps{b}")
                for cb in range(2):
                    nc.tensor.matmul(
                        hid_ps,
                        fc1wT_sb[:, cb * MID : (cb + 1) * MID],
                        pooled[cb],
                        start=(cb == 0),
                        stop=False,
                    )
                nc.tensor.matmul(
                    hid_ps, fc1b_sb, ones_sb[:, 0:1], start=False, stop=True
                )

                hid_sb = constp.tile([MID + 1, 1], f32, tag=f"hid{b}", name=f"hid_sb{b}")
                nc.gpsimd.memset(hid_sb[MID : MID + 1, :], 1.0)
                nc.scalar.activation(hid_sb[0:MID, :], hid_ps, ACTF.Relu)

                lg_ps = smallps.tile([1, K], f32, tag="small", name=f"lg_ps{b}")
                nc.tensor.matmul(lg_ps, hid_sb, fc2aug_sb, start=True, stop=True)

                # softmax without max-shift (|logits|/30 is small) and with the
                # denominator accumulated inside the Exp activation
                ex = constp.tile([1, K], f32, tag=f"ex{b}", name=f"ex{b}")
                sm = constp.tile([1, 1], f32, tag=f"sm{b}", name=f"sm{b}")
                nc.scalar.activation(ex, lg_ps, ACTF.Exp, accum_out=sm)
                rc = constp.tile([1, 1], f32, tag=f"rc{b}", name=f"rc{b}")
                nc.vector.reciprocal(rc, sm)
                attn = constp.tile([1, K], f32, tag=f"at{b}", name=f"attn{b}")
                nc.vector.tensor_scalar_mul(attn, ex, rc)

                # broadcast attn row to all 128 partitions (gpsimd ISA op)
                attn_bc = constp.tile([128, K], f32, tag=f"abc{b}", name=f"attn_bc{b}")
                nc.gpsimd.partition_broadcast(attn_bc, attn)

                # mixed transposed weights: [128c, pos*256 + o].  Accumulate the
                # first K-1 terms into an fp32 scratch (single bf16 rounding at
                # the end instead of one per step).  Force the cb0 chain to
                # finish before cb1 starts so cb0 conv matmuls unblock at
                # half-time instead of both chains finishing together.
                # Four sub-chains (cb x free-half), strictly ordered on DVE so
                # each finished half releases its conv matmuls (subtile deps)
                # at quarter-points instead of everything at the end.
                HALF = 5 * O  # pos 0-4 first (covers the earliest-emitted MMs)
                prev_last = None
                for cb in range(2):
                    sc = wmixp.tile([128, NPOS * O], f32, tag="mixsc", bufs=2, name=f"mixsc{b}{cb}")
                    wm = wmixp.tile(
                        [128, NPOS * O], bf16, tag=f"wm{b}{cb}", name=f"wmix{b}{cb}"
                    )
                    for lo, hi in ((0, HALF), (HALF, NPOS * O)):
                        first = nc.vector.tensor_scalar_mul(
                            sc[:, lo:hi], wdyn[0][cb][:, lo:hi], attn_bc[:, 0:1]
                        )
                        if prev_last is not None:
                            add_dep_helper(
                                first.ins,
                                prev_last.ins,
                                sync=False,
                                reason="mix sub-chain ordering",
                            )
                        for k in range(1, K - 1):
                            nc.vector.scalar_tensor_tensor(
                                sc[:, lo:hi],
                                wdyn[k][cb][:, lo:hi],
                                attn_bc[:, k : k + 1],
                                sc[:, lo:hi],
                                op0=ALU.mult,
                                op1=ALU.add,
                            )
                        prev_last = nc.vector.scalar_tensor_tensor(
                            wm[:, lo:hi],
                            wdyn[K - 1][cb][:, lo:hi],
                            attn_bc[:, K - 1 : K],
                            sc[:, lo:hi],
                            op0=ALU.mult,
                            op1=ALU.add,
                        )
                    wmix[b][cb] = wm

            attn_and_mix(0)
            load_x(1)
            attn_and_mix(1)

            # ---------------- main conv ----------------
            def wsl(b, cb, pos, ob):
                off = pos * O + ob * 128
                return wmix[b][cb][:, off : off + 128]

            for b in range(NB):
                for ob in range(2):
                    # border corrections: [128o, side*64 + y]
                    corr = corrps.tile([128, 128], f32, tag="corr", name=f"corr{b}{ob}")
                    for side, dxv in ((0, 0), (1, 2)):
                        i = 0
                        for cb in range(2):
                            for dy in range(KS):
                                s = dy * W + (0 if side == 0 else ROW0)
                                rhs = xpad[b][cb][:, s : s + (H - 1) * W + 1 : W]
                                nc.tensor.matmul(
                                    corr[:, side * 64 : side * 64 + 64],
                                    wsl(b, cb, dy * KS + dxv, ob),
                                    rhs,
                                    start=(i == 0),
                                    stop=(i == 5),
                                )
                                i += 1

                    for rg in range(8):
                        y0 = rg * 8
                        cps = convps.tile([128, 512], f32, tag="conv", name=f"cps{b}{ob}{rg}")
                        i = 0
                        for cb in range(2):
                            for dy in range(KS):
                                for dx in range(KS):
                                    s = (y0 + dy) * W + dx
                                    nc.tensor.matmul(
                                        cps,
                                        wsl(b, cb, dy * KS + dx, ob),
                                        xpad[b][cb][:, s : s + 512],
                                        start=(i == 0),
                                        stop=(i == 17),
                                    )
                                    i += 1
                        osb = osbp.tile([128, 512], f32, tag="osb", name=f"osb{b}{ob}{rg}")
                        nc.scalar.copy(osb, cps)
                        # subtract wrap-around bleed on columns 0 and 63
                        ov = osb.rearrange("m (y x) -> m y x", x=W)[:, :, 0 : W : W - 1]
                        cv = corr.rearrange("m (s y) -> m y s", s=2)[:, y0 : y0 + 8, :]
                        nc.vector.tensor_sub(ov, ov, cv)
                        nc.sync.dma_start(
                            out_d[b, ob * 128 : (ob + 1) * 128, y0 : y0 + 8, :],
                            osb.rearrange("m (y x) -> m y x", x=W),
                        )

    nc.compile()
    return nc


def get_nc():
    if "nc" not in _CACHE:
        _CACHE["nc"] = _build_nc()
    return _CACHE["nc"]


def prep_inputs(x, w_dyn, fc1_w, fc1_b, fc2_w, fc2_b):
    """Host-side layout prep + batch sharding -> per-core input maps."""
    import ml_dtypes

    bf16 = ml_dtypes.bfloat16
    w_dynT = np.ascontiguousarray(
        np.transpose(np.asarray(w_dyn, np.float32), (0, 3, 4, 2, 1)).reshape(K, NPOS, C, O)
    ).astype(bf16)
    fc1wT = np.ascontiguousarray(np.asarray(fc1_w, np.float32).T) / float(H * W)
    fc1b = np.ascontiguousarray(np.asarray(fc1_b, np.float32).reshape(1, MID))
    fc2aug = np.ascontiguousarray(
        np.vstack([np.asarray(fc2_w, np.float32).T, np.asarray(fc2_b, np.float32)[None, :]])
        * INV_DELTA
    )
    x = np.asarray(x, np.float32).astype(bf16)
    in_maps = []
    for core in range(NCORES):
        in_maps.append(
            {
                "x": np.ascontiguousarray(x[core * NB : (core + 1) * NB]),
                "wdynT": w_dynT,
                "fc1wT": fc1wT,
                "fc1b": fc1b,
                "fc2aug": fc2aug,
            }
        )
    return in_maps


def kernel(x, w_dyn, fc1_w, fc1_b, fc2_w, fc2_b):
    from concourse.bass_utils import run_bass_kernel_spmd

    nc = get_nc()
    in_maps = prep_inputs(x, w_dyn, fc1_w, fc1_b, fc2_w, fc2_b)
    res = run_bass_kernel_spmd(nc, in_maps, core_ids=list(range(NCORES)))
    return np.concatenate([r["out"] for r in res.results], axis=0)
